# revision 3
# baseline (speedup 1.0000x reference)
"""Trainium2 Bass kernel for nn_MinBlcokScan: 4 grouped 1-D cross-correlations.

Math (reference): x = batch_x.reshape(B, 32, L). For each group g of 4,
channels rel_g = [8g..8g+7] are convolved ('same', zero pad 2/2) with
kernels_g [4, 8, 5], producing out[:, 4g+o, :]; the 16 output channels are
concatenated and flattened to [B, 16*L].

Strategy: pure data parallel over batch (4 samples per core) plus a
polyphase-2 reformulation that packs two L-positions per streamed PE
column, cutting TensorEngine column count from 5L to 3L.

Host-side marshalling (free for the device):
  x is zero-padded by 2 each side and parity-interleaved:
    x_i[(s, c, p), m] = x_pad[s, c, 2m + p],  m in [0, L/2+2)
  so one SBUF partition column m carries both parities for 2 samples x 32
  channels = 128 contraction rows. The conv becomes 3 PSUM-accumulated
  matmuls per output tile, with window offsets d in {-1,0,+1}:
    y[s, o, 2m+r] = sum_d (W_d.T @ x_i[:, m+d])[(s,o,r)]
    W_d[(s,c,p), (s,o,r)] = ker[o, c, t],  t = 2d + p + 2 - r  (valid t only)
  block-diagonal over the 2 samples of a pass; 2 passes cover 4 samples.
  Output is produced parity-interleaved y_i[(s, o, r), m] = y[s, o, 2m+r]
  and de-interleaved on the host.

Matmuls run in float32r (full-rate fp32 PE mode, fp32 PSUM accumulate),
so accuracy is ~1e-4 relative.
"""

import numpy as np
from contextlib import ExitStack

import concourse.bass as bass
import concourse.bacc as bacc
import concourse.mybir as mybir
import concourse.tile as tile
from concourse.bass_utils import run_bass_kernel_spmd

D = 32          # input channels
L_FULL = 65536  # sequence length
W = 5           # conv window
PAD = 2         # left zero-pad ('same')
B = 32          # batch
N_CORES = 8
S = 4           # samples per core
NSUB = 512      # matmul moving free dim == one fp32 PSUM bank
ND = 3          # window offsets d in {-1, 0, 1}
F32 = mybir.dt.float32
F32R = mybir.dt.float32r


def build_program(L=L_FULL, blk_m=2048, reps=1, variant="full"):
    """Build the single-core SPMD Bass program (same program on all cores).

    blk_m: per-block m-columns (= blk_m*2 L positions).
    reps > 1 wraps the body in a hardware For_i loop (steady-state timing).
    variant: "full" | "dma" (loads+stores only) | "pe" (loads+matmuls only)
    """
    M = L // 2  # m-columns total
    assert M % blk_m == 0 and blk_m % NSUB == 0
    nblk = M // blk_m
    nq = blk_m // NSUB

    nc = bacc.Bacc(trn_type="TRN2", target_bir_lowering=False, debug=False)
    x = nc.dram_tensor("x", [2 * 128, M + 2], F32R, kind="ExternalInput").ap()
    w = nc.dram_tensor("w", [ND, 128, 64], F32R, kind="ExternalInput").ap()
    y = nc.dram_tensor("y", [128, M], F32, kind="ExternalOutput").ap()

    with tile.TileContext(nc) as tc, ExitStack() as ctx:
        xp = ctx.enter_context(tc.tile_pool(name="xp", bufs=4))
        wp = ctx.enter_context(tc.tile_pool(name="wp", bufs=1))
        op = ctx.enter_context(tc.tile_pool(name="op", bufs=3))
        pp = ctx.enter_context(tc.tile_pool(name="pp", bufs=8, space="PSUM"))

        # Load the 3 offset-weight matrices once: wt[:, d*64 + mcol] = w[d, :, mcol]
        wt = wp.tile([128, ND * 64], F32R)
        nc.sync.dma_start(
            wt[:].rearrange("p (d m) -> p d m", d=ND),
            w.rearrange("d p m -> p d m"),
        )

        if reps > 1:
            loop_cm = tc.For_i(
                0, reps, 1,
                hint_engines=(mybir.EngineType.PE, mybir.EngineType.DVE,
                              mybir.EngineType.SP, mybir.EngineType.Activation),
            )
            ctx.enter_context(loop_cm)

        for b in range(nblk):
            m0 = b * blk_m
            ot = None
            if variant != "pe":
                ot = op.tile([128, blk_m], F32)
            if variant == "dma":
                nc.vector.memset(ot[:], 0.0)

            for ps in range(2):  # sample-pair pass: samples (2ps, 2ps+1)
                xt = xp.tile([128, blk_m + 2], F32R)
                nc.sync.dma_start(xt[:], x[128 * ps : 128 * (ps + 1), m0 : m0 + blk_m + 2])

                if variant == "dma":
                    continue
                for q in range(nq):
                    pt = pp.tile([64, NSUB], F32)
                    for d in range(ND):
                        nc.tensor.matmul(
                            pt[:],
                            wt[:, d * 64 : (d + 1) * 64],
                            xt[:, q * NSUB + d : q * NSUB + d + NSUB],
                            start=(d == 0),
                            stop=(d == ND - 1),
                        )
                    if variant == "full":
                        # partition-shifted PSUM->SBUF copy (ps=1 -> 64:128)
                        nc.vector.tensor_copy(
                            ot[ps * 64 : (ps + 1) * 64, q * NSUB : (q + 1) * NSUB],
                            pt[:],
                        )

            if variant != "pe":
                nc.scalar.dma_start(y[:, m0 : m0 + blk_m], ot[:])
    nc.compile()
    return nc


def build_weights(kernels):
    """W_d [3, 128, 64]: W_d[(s,c,p), (s,o,r)] = ker_g[o, c, t], t = 2d+p+2-r.

    s in {0,1} is the sample within a pass (block-diagonal), c channel,
    p source parity, o output channel (16 = 4 groups x 4), r output parity.
    """
    Wd = np.zeros((ND, 128, 64), np.float32)
    for g, ker in enumerate(kernels):  # ker [4, 8, 5]
        for o_in_g in range(4):
            o = 4 * g + o_in_g
            for c_in_g in range(8):
                c = 8 * g + c_in_g
                for r in range(2):
                    for t in range(W):
                        dd = (r + t - 2) >> 1  # floor((r+t-2)/2)
                        p = (r + t - 2) - 2 * dd
                        assert -1 <= dd <= 1
                        for s in range(2):
                            Wd[dd + 1, s * 64 + c * 2 + p, s * 32 + o * 2 + r] = \
                                ker[o_in_g, c_in_g, t]
    return Wd


def interleave_x(x4, L):
    """[4, 32, L] -> [256, L/2+2]: row (s*64 + c*2 + p), col m = x_pad[s,c,2m+p]."""
    xp = np.zeros((4, D, L + 4), np.float32)
    xp[:, :, 2 : L + 2] = x4
    xi = xp.reshape(4, D, (L + 4) // 2, 2).transpose(0, 1, 3, 2)  # s, c, p, m
    return np.ascontiguousarray(xi.reshape(256, (L + 4) // 2))


def deinterleave_y(yi, L):
    """[128, L/2] -> [64, L]: yi[s*32+o*2+r, m] = y[s*16+o, 2m+r]."""
    t = yi.reshape(4, 16, 2, L // 2).transpose(0, 1, 3, 2)  # s, o, m, r
    return np.ascontiguousarray(t.reshape(64, L))


_program_cache = {}

# Set PROFILE=True (e.g. from a test harness) to capture an NTFF profile;
# the BassKernelResults lands in LAST_RESULT.
PROFILE = False
PROFILE_TMPDIR = None
LAST_RESULT = None


def kernel(batch_x, kernels0, kernels1, kernels2, kernels3):
    global LAST_RESULT
    batch_x = np.asarray(batch_x)
    kernels = [np.asarray(k) for k in (kernels0, kernels1, kernels2, kernels3)]
    Wd = build_weights(kernels)

    if "nc" not in _program_cache:
        _program_cache["nc"] = build_program()
    nc = _program_cache["nc"]

    in_maps = [
        {
            "x": interleave_x(
                batch_x[S * k : S * (k + 1)].reshape(S, D, L_FULL), L_FULL
            ),
            "w": Wd,
        }
        for k in range(N_CORES)
    ]
    res = run_bass_kernel_spmd(
        nc, in_maps, list(range(N_CORES)), trace=PROFILE, tmpdir=PROFILE_TMPDIR
    )
    LAST_RESULT = res
    ys = [deinterleave_y(res.results[k]["y"], L_FULL) for k in range(N_CORES)]
    return np.concatenate(ys, axis=0).reshape(B, 16 * L_FULL)



# revision 4
# speedup vs baseline: 1.3105x; 1.3105x over previous
"""Trainium2 Bass kernel for nn_MinBlcokScan: 4 grouped 1-D cross-correlations.

Math (reference): x = batch_x.reshape(B, 32, L). For each group g of 4,
channels rel_g = [8g..8g+7] are convolved ('same', zero pad 2/2) with
kernels_g [4, 8, 5], producing out[:, 4g+o, :]; the 16 output channels are
concatenated and flattened to [B, 16*L].

Strategy: pure data parallel over batch (4 samples per core). The conv is
memory-bound (headroom vs compute ~20x), so the kernel streams x and y in
fp16 (host-side cast is free; rel-err budget 2e-2 vs fp16's ~2e-4) and uses
a *shifted* polyphase-4 reformulation so each PSUM tile needs only 2
PSUM-accumulated matmuls (not 5 naive / 3 for centered polyphase):

Host-side marshalling (free for the device):
  x_pad[c, i] = x[c, i-2] (zero pad 2 both sides), and per sample
    x_i[(c, p), m] = x_pad[c, 4m + p],  p in [0,4), m in [0, L/4 + 1)
  i.e. column m carries the 4 input positions 4m-2 .. 4m+1. Then
    y[o, 4m+r] = sum_d (W_d.T @ x_i[:, m+d])[(o,r)],  d in {0, 1}
    W_d[(c,p), (o,r)] = ker[o, c, t],  t = 4d + p - r  (when 0 <= t <= 4)
  because output 4m+r's receptive field is positions 4m+r-2 .. 4m+r+2,
  which the -2 shift places entirely inside columns m and m+1.
  Output is produced parity-interleaved y_i[(o, r), m] = y[o, 4m+r] in
  fp16 and de-interleaved/upcast on the host.

Contraction is 32 ch x 4 parities = 128 (full PE height); outputs are
16 ch x 4 parities = 64 partitions. PE streams 2 x L/4 columns per sample
= 2L columns per core total (~55 us) while DMA moves 16.8 MB in + 8.4 MB
out = 25.2 MB (~70 us at ~355 GB/s/core) -> DMA-bound.
"""

import numpy as np
from contextlib import ExitStack

import concourse.bass as bass
import concourse.bacc as bacc
import concourse.mybir as mybir
import concourse.tile as tile
from concourse.bass_utils import run_bass_kernel_spmd

D = 32          # input channels
L = 65536       # sequence length
W = 5           # conv window
B = 32          # batch
N_CORES = 8
S = 4           # samples per core
P = 4           # polyphase factor
M = L // P      # 16384 output columns per sample
MC = M + 1      # stored x columns per sample (one halo column)
ND = 2          # window offsets d in {0, 1}
NSUB = 512      # one fp32 PSUM bank = 512 columns
F16 = mybir.dt.float16
F32 = mybir.dt.float32


def build_program(blk=2048, variant="full"):
    """Build the single-core SPMD Bass program (same program on all cores).

    blk: per-block m-columns (= blk*4 L positions); must divide M and be a
    multiple of NSUB.
    variant: "full" | "dma" (loads+stores only) | "pe" (loads+matmuls only)
    """
    assert M % blk == 0 and blk % NSUB == 0
    nblk = M // blk
    nq = blk // NSUB

    nc = bacc.Bacc(trn_type="TRN2", target_bir_lowering=False, debug=False)
    x = nc.dram_tensor("x", [128, S * MC], F16, kind="ExternalInput").ap()
    w = nc.dram_tensor("w", [128, ND * 64], F16, kind="ExternalInput").ap()
    y = nc.dram_tensor("y", [64, S * M], F16, kind="ExternalOutput").ap()

    with tile.TileContext(nc) as tc, ExitStack() as ctx:
        xp = ctx.enter_context(tc.tile_pool(name="xp", bufs=4))
        wp = ctx.enter_context(tc.tile_pool(name="wp", bufs=1))
        op = ctx.enter_context(tc.tile_pool(name="op", bufs=3))
        # [64, blk] fp32 = blk/512 banks per tile; 2 tiles fill all 8 banks
        pp = ctx.enter_context(tc.tile_pool(name="pp", bufs=2, space="PSUM"))

        wt = wp.tile([128, ND * 64], F16)
        nc.sync.dma_start(wt[:], w)

        for s in range(S):
            for b in range(nblk):
                x0 = s * MC + b * blk
                xt = xp.tile([128, blk + 1], F16)
                nc.sync.dma_start(xt[:], x[:, x0 : x0 + blk + 1])

                ot = None
                if variant != "pe":
                    ot = op.tile([64, blk], F16)
                if variant == "dma":
                    nc.vector.memset(ot[:], 0.0)
                else:
                    pt = pp.tile([64, blk], F32)
                    for q in range(nq):
                        for d in range(ND):
                            nc.tensor.matmul(
                                pt[:, q * NSUB : (q + 1) * NSUB],
                                wt[:, d * 64 : (d + 1) * 64],
                                xt[:, q * NSUB + d : q * NSUB + d + NSUB],
                                start=(d == 0),
                                stop=(d == ND - 1),
                            )
                    if variant == "full":
                        nc.vector.tensor_copy(ot[:], pt[:])

                if variant != "pe":
                    y0 = s * M + b * blk
                    nc.scalar.dma_start(y[:, y0 : y0 + blk], ot[:])
    nc.compile()
    return nc


def build_weights(kernels):
    """W [128, 2*64]: W[(c,p), d*64 + (o,r)] = ker_g[o, c, t], t = 4d+p-r."""
    Wd = np.zeros((ND, 128, 64), np.float32)
    for g, ker in enumerate(kernels):  # ker [4, 8, 5]
        for oo in range(4):
            o = 4 * g + oo
            for cc in range(8):
                c = 8 * g + cc
                for r in range(P):
                    for p in range(P):
                        for d in range(ND):
                            t = P * d + p - r
                            if 0 <= t < W:
                                Wd[d, c * P + p, o * P + r] = ker[oo, cc, t]
    return np.ascontiguousarray(
        np.concatenate([Wd[0], Wd[1]], axis=1)
    ).astype(np.float16)


def interleave_x(x1, dtype=np.float16):
    """[32, L] -> [128, L/4+1] fp16: row (c*4+p), col m = x_pad[c, 4m+p]."""
    xp = np.zeros((D, L + P), np.float32)
    xp[:, 2 : L + 2] = x1
    xi = xp.reshape(D, MC, P).transpose(0, 2, 1)  # c, p, m
    return np.ascontiguousarray(xi.reshape(D * P, MC).astype(dtype))


def deinterleave_y(yi):
    """[64, S*M] fp16 -> [S*16, L] fp32: yi[o*4+r, s*M+m] = y[s*16+o, 4m+r]."""
    t = yi.reshape(16, P, S, M).transpose(2, 0, 3, 1)  # s, o, m, r
    return np.ascontiguousarray(t.astype(np.float32).reshape(S * 16, L))


_program_cache = {}

# Set PROFILE=True (e.g. from a test harness) to capture an NTFF profile;
# the BassKernelResults lands in LAST_RESULT.
PROFILE = False
PROFILE_TMPDIR = None
LAST_RESULT = None


def kernel(batch_x, kernels0, kernels1, kernels2, kernels3):
    global LAST_RESULT
    batch_x = np.asarray(batch_x)
    kernels = [np.asarray(k) for k in (kernels0, kernels1, kernels2, kernels3)]
    Wd = build_weights(kernels)

    if "nc" not in _program_cache:
        _program_cache["nc"] = build_program()
    nc = _program_cache["nc"]

    in_maps = []
    for k in range(N_CORES):
        xs = [
            interleave_x(batch_x[S * k + s].reshape(D, L)) for s in range(S)
        ]
        in_maps.append({"x": np.concatenate(xs, axis=1), "w": Wd})

    res = run_bass_kernel_spmd(
        nc, in_maps, list(range(N_CORES)), trace=PROFILE, tmpdir=PROFILE_TMPDIR
    )
    LAST_RESULT = res
    ys = [deinterleave_y(res.results[k]["y"]) for k in range(N_CORES)]
    return np.concatenate(ys, axis=0).reshape(B, 16 * L)


# revision 5
# speedup vs baseline: 1.5171x; 1.1576x over previous
"""Trainium2 Bass kernel for nn_MinBlcokScan: 4 grouped 1-D cross-correlations.

Math (reference): x = batch_x.reshape(B, 32, L). For each group g of 4,
channels rel_g = [8g..8g+7] are convolved ('same', zero pad 2/2) with
kernels_g [4, 8, 5], producing out[:, 4g+o, :]; the 16 output channels are
concatenated and flattened to [B, 16*L].

Strategy: pure data parallel over batch (4 samples per core). The conv is
memory-bound, so x and y stream as fp16 (host-side cast is free; rel-err
budget 2e-2 vs fp16's ~3e-4) with a split-parity polyphase-8 layout that
fills the full 128x128 PE array:

Host-side marshalling (free for the device):
  x_pad[c, i] = x[c, i-2] (zero pad, length L+8), split into even/odd
  half-columns of 4 consecutive positions:
    x_e[(c,p), m] = x_pad[c, 8m + p]       p in [0,4), m in [0, L/8]
    x_o[(c,p), m] = x_pad[c, 8m + 4 + p]   p in [0,4), m in [0, L/8)
  Output column m carries 8 positions (r' in [0,8)); its receptive field
  [8m-2, 8m+9] is exactly x_e[:, m] U x_o[:, m] U x_e[:, m+1], so each
  PSUM tile accumulates 3 matmuls with 128-partition outputs:
    y[o, 8m+r'] = (We0.T x_e[:,m] + Wo.T x_o[:,m] + We1.T x_e[:,m+1])[(o,r')]
    We0[(c,p),(o,r')] = ker[o,c, p-r']      (offset v = p-2  in [-2,2))
    Wo [(c,p),(o,r')] = ker[o,c, p+4-r']    (offset v = p+2  in [2,6))
    We1[(c,p),(o,r')] = ker[o,c, p+8-r']    (offset v = p+6  in [6,10))
  (entries with tap index outside [0,5) are zero). Output is produced
  parity-interleaved y_i[(o,r'), m] = y[o, 8m+r'] in fp16 and
  de-interleaved/upcast on the host.

Per core: PE streams 3 * L/8 * 4 = 1.5L columns (~41 us at 2.4 GHz), DMA
moves 16.8 MB in + 8.4 MB out = 25.2 MB (~70 us at ~355 GB/s) -> DMA-bound.
PSUM->SBUF drains are full-width [128, 2048] copies alternating between
the DVE and Activation engines.
"""

import numpy as np
from contextlib import ExitStack

import concourse.bass as bass
import concourse.bacc as bacc
import concourse.mybir as mybir
import concourse.tile as tile
from concourse.bass_utils import run_bass_kernel_spmd

D = 32          # input channels
L = 65536       # sequence length
W = 5           # conv window
B = 32          # batch
N_CORES = 8
S = 4           # samples per core
MO = L // 8     # 8192 output columns per sample
ME = MO + 1     # stored even x columns per sample (one halo column)
MC = ME + MO    # per-sample x segment: [x_e | x_o] = 16385 columns
NSUB = 512      # one fp32 PSUM bank = 512 columns at 128 partitions
F16 = mybir.dt.float16
F32 = mybir.dt.float32


def build_program(blk=2048, variant="full"):
    """Build the single-core SPMD Bass program (same program on all cores).

    blk: output m-columns per block (= blk*8 L positions); must divide MO
    and be a multiple of NSUB.
    variant: "full" | "dma" (loads+stores only) | "pe" (loads+matmuls only)
    """
    assert MO % blk == 0 and blk % NSUB == 0
    nblk = MO // blk
    nq = blk // NSUB

    nc = bacc.Bacc(trn_type="TRN2", target_bir_lowering=False, debug=False)
    x = nc.dram_tensor("x", [128, S * MC], F16, kind="ExternalInput").ap()
    w = nc.dram_tensor("w", [128, 3 * 128], F16, kind="ExternalInput").ap()
    y = nc.dram_tensor("y", [128, S * MO], F16, kind="ExternalOutput").ap()

    with tile.TileContext(nc) as tc, ExitStack() as ctx:
        xep = ctx.enter_context(tc.tile_pool(name="xep", bufs=5))
        xop = ctx.enter_context(tc.tile_pool(name="xop", bufs=5))
        wp = ctx.enter_context(tc.tile_pool(name="wp", bufs=1))
        op = ctx.enter_context(tc.tile_pool(name="op", bufs=4))
        # [128, blk] fp32 = blk/512 banks per tile; 2 tiles fill all 8 banks
        pp = ctx.enter_context(tc.tile_pool(name="pp", bufs=2, space="PSUM"))

        wt = wp.tile([128, 3 * 128], F16)
        nc.sync.dma_start(wt[:], w)

        copy_engines = [nc.vector.tensor_copy, nc.scalar.copy]
        blk_idx = 0
        for s in range(S):
            e0 = s * MC            # even columns base
            o0 = s * MC + ME       # odd columns base
            for b in range(nblk):
                m0 = b * blk
                xe = xep.tile([128, blk + 1], F16)
                nc.sync.dma_start(xe[:], x[:, e0 + m0 : e0 + m0 + blk + 1])
                xo = xop.tile([128, blk], F16)
                nc.sync.dma_start(xo[:], x[:, o0 + m0 : o0 + m0 + blk])

                ot = None
                if variant != "pe":
                    ot = op.tile([128, blk], F16)
                if variant == "dma":
                    nc.vector.memset(ot[:], 0.0)
                else:
                    pt = pp.tile([128, blk], F32)
                    for q in range(nq):
                        psl = pt[:, q * NSUB : (q + 1) * NSUB]
                        nc.tensor.matmul(
                            psl, wt[:, 0:128],
                            xe[:, q * NSUB : q * NSUB + NSUB],
                            start=True, stop=False,
                        )
                        nc.tensor.matmul(
                            psl, wt[:, 128:256],
                            xo[:, q * NSUB : q * NSUB + NSUB],
                            start=False, stop=False,
                        )
                        nc.tensor.matmul(
                            psl, wt[:, 256:384],
                            xe[:, q * NSUB + 1 : q * NSUB + 1 + NSUB],
                            start=False, stop=True,
                        )
                    if variant == "full":
                        copy_engines[blk_idx % 2](ot[:], pt[:])

                if variant != "pe":
                    y0 = s * MO + m0
                    nc.scalar.dma_start(y[:, y0 : y0 + blk], ot[:])
                blk_idx += 1
    nc.compile()
    return nc


def build_weights(kernels):
    """W [128, 3*128]: W[(c,p), j*128 + (o,r')] = ker_g[o, c, t] where
    t = p - r' (j=0, x_e), p + 4 - r' (j=1, x_o), p + 8 - r' (j=2, x_e+1)."""
    Wd = np.zeros((3, 128, 128), np.float32)
    for g, ker in enumerate(kernels):  # ker [4, 8, 5]
        for oo in range(4):
            o = 4 * g + oo
            for cc in range(8):
                c = 8 * g + cc
                for rp in range(8):
                    for p in range(4):
                        for j in range(3):
                            t = 4 * j + p - rp
                            if 0 <= t < W:
                                Wd[j, c * 4 + p, o * 8 + rp] = ker[oo, cc, t]
    return np.ascontiguousarray(
        np.concatenate([Wd[0], Wd[1], Wd[2]], axis=1)
    ).astype(np.float16)


def interleave_x(x1, dtype=np.float16):
    """[32, L] -> [128, MC]: per-sample [x_e | x_o] split-parity layout."""
    xp = np.zeros((D, L + 8), np.float32)
    xp[:, 2 : L + 2] = x1
    xr = xp.reshape(D, ME, 8)  # ME * 8 = L + 8
    xe = xr[:, :, 0:4].transpose(0, 2, 1).reshape(D * 4, ME)
    xo = xr[:, :MO, 4:8].transpose(0, 2, 1).reshape(D * 4, MO)
    return np.ascontiguousarray(
        np.concatenate([xe, xo], axis=1).astype(dtype)
    )


def deinterleave_y(yi):
    """[128, S*MO] fp16 -> [S*16, L] fp32: yi[o*8+r', s*MO+m] = y[s,o,8m+r']."""
    t = yi.reshape(16, 8, S, MO).transpose(2, 0, 3, 1)  # s, o, m, r'
    return np.ascontiguousarray(t.astype(np.float32).reshape(S * 16, L))


_program_cache = {}

# Set PROFILE=True (e.g. from a test harness) to capture an NTFF profile;
# the BassKernelResults lands in LAST_RESULT.
PROFILE = False
PROFILE_TMPDIR = None
LAST_RESULT = None


def kernel(batch_x, kernels0, kernels1, kernels2, kernels3):
    global LAST_RESULT
    batch_x = np.asarray(batch_x)
    kernels = [np.asarray(k) for k in (kernels0, kernels1, kernels2, kernels3)]
    Wd = build_weights(kernels)

    if "nc" not in _program_cache:
        _program_cache["nc"] = build_program()
    nc = _program_cache["nc"]

    in_maps = []
    for k in range(N_CORES):
        xs = [
            interleave_x(batch_x[S * k + s].reshape(D, L)) for s in range(S)
        ]
        in_maps.append({"x": np.concatenate(xs, axis=1), "w": Wd})

    res = run_bass_kernel_spmd(
        nc, in_maps, list(range(N_CORES)), trace=PROFILE, tmpdir=PROFILE_TMPDIR
    )
    LAST_RESULT = res
    ys = [deinterleave_y(res.results[k]["y"]) for k in range(N_CORES)]
    return np.concatenate(ys, axis=0).reshape(B, 16 * L)


# revision 6
# speedup vs baseline: 1.7737x; 1.1692x over previous
"""Trainium2 Bass kernel for nn_MinBlcokScan: 4 grouped 1-D cross-correlations.

Math (reference): x = batch_x.reshape(B, 32, L). For each group g of 4,
channels rel_g = [8g..8g+7] are convolved ('same', zero pad 2/2) with
kernels_g [4, 8, 5], producing out[:, 4g+o, :]; the 16 output channels are
concatenated and flattened to [B, 16*L].

Strategy: pure data parallel over batch (4 samples per core). The conv is
memory-bound, so x and y stream as fp16 (host-side cast is free; rel-err
budget 2e-2 vs fp16's ~3e-4) with a split-parity polyphase-8 layout that
fills the full 128x128 PE array:

Host-side marshalling (free for the device):
  x_pad[c, i] = x[c, i-2] (zero pad, length L+8), split into even/odd
  half-columns of 4 consecutive positions:
    x_e[(c,p), m] = x_pad[c, 8m + p]       p in [0,4), m in [0, L/8]
    x_o[(c,p), m] = x_pad[c, 8m + 4 + p]   p in [0,4), m in [0, L/8)
  Output column m carries 8 positions (r' in [0,8)); its receptive field
  [8m-2, 8m+9] is exactly x_e[:, m] U x_o[:, m] U x_e[:, m+1], so each
  PSUM tile accumulates 3 matmuls with 128-partition outputs:
    y[o, 8m+r'] = (We0.T x_e[:,m] + Wo.T x_o[:,m] + We1.T x_e[:,m+1])[(o,r')]
    We0[(c,p),(o,r')] = ker[o,c, p-r']      (offset v = p-2  in [-2,2))
    Wo [(c,p),(o,r')] = ker[o,c, p+4-r']    (offset v = p+2  in [2,6))
    We1[(c,p),(o,r')] = ker[o,c, p+8-r']    (offset v = p+6  in [6,10))
  (entries with tap index outside [0,5) are zero). Output is produced
  parity-interleaved y_i[(o,r'), m] = y[o, 8m+r'] in fp16 and
  de-interleaved/upcast on the host.

Per core: PE streams 3 * L/8 * 4 = 1.5L columns (~41 us at 2.4 GHz), DMA
moves 16.8 MB in + 8.4 MB out = 25.2 MB (~70 us at ~355 GB/s) -> DMA-bound.
PSUM->SBUF drains are full-width [128, 2048] copies alternating between
the DVE and Activation engines.
"""

import numpy as np
from contextlib import ExitStack

import concourse.bass as bass
import concourse.bacc as bacc
import concourse.mybir as mybir
import concourse.tile as tile
from concourse.bass_utils import run_bass_kernel_spmd

D = 32          # input channels
L = 65536       # sequence length
W = 5           # conv window
B = 32          # batch
N_CORES = 8
S = 4           # samples per core
MO = L // 8     # 8192 output columns per sample
ME = MO + 1     # stored even x columns per sample (one halo column)
MC = ME + MO    # per-sample x segment: [x_e | x_o] = 16385 columns
NSUB = 512      # one fp32 PSUM bank = 512 columns at 128 partitions
F16 = mybir.dt.float16
F32 = mybir.dt.float32


def build_program(variant="full"):
    """Build the single-core SPMD Bass program (same program on all cores).

    Work unit: "super-block" of SB=4096 output columns (one xe/xo DMA pair
    with 8 KB/partition descriptors), computed as sub-blocks of 2048 cols
    (one [128, 2048] PSUM tile = 4 banks, weight-outer: 3 stationary loads
    x 4 matmuls). The final super-block tapers into 2048/1024/512/512
    sub-blocks with per-sub stores to shorten the pipeline drain.
    variant: "full" | "dma" (loads+stores only) | "pe" (loads+matmuls only)
    """
    SB = 4096
    nsb = MO // SB  # super-blocks per sample

    nc = bacc.Bacc(trn_type="TRN2", target_bir_lowering=False, debug=False)
    x = nc.dram_tensor("x", [128, S * MC], F16, kind="ExternalInput").ap()
    w = nc.dram_tensor("w", [128, 3 * 128], F16, kind="ExternalInput").ap()
    y = nc.dram_tensor("y", [128, S * MO], F16, kind="ExternalOutput").ap()

    with tile.TileContext(nc) as tc, ExitStack() as ctx:
        xep = ctx.enter_context(tc.tile_pool(name="xep", bufs=3))
        xop = ctx.enter_context(tc.tile_pool(name="xop", bufs=3))
        wp = ctx.enter_context(tc.tile_pool(name="wp", bufs=1))
        op = ctx.enter_context(tc.tile_pool(name="op", bufs=3))
        # [128, 2048] fp32 = 4 banks per tile; 2 tiles fill all 8 banks
        pp = ctx.enter_context(tc.tile_pool(name="pp", bufs=2, space="PSUM"))

        wt = wp.tile([128, 3 * 128], F16)
        nc.sync.dma_start(wt[:], w)

        copy_engines = [nc.vector.tensor_copy, nc.scalar.copy]
        sub_idx = 0
        for s in range(S):
            e0 = s * MC            # even columns base
            o0 = s * MC + ME       # odd columns base
            for b in range(nsb):
                m0 = b * SB
                last = (s == S - 1) and (b == nsb - 1)
                # taper the final super-block so the drain chain is short
                subs = [2048, 1024, 512, 512] if last else [2048, 2048]

                xe = xep.tile([128, SB + 1], F16)
                nc.sync.dma_start(xe[:], x[:, e0 + m0 : e0 + m0 + SB + 1])
                xo = xop.tile([128, SB], F16)
                nc.sync.dma_start(xo[:], x[:, o0 + m0 : o0 + m0 + SB])

                ot = None
                if variant != "pe" and not last:
                    ot = op.tile([128, SB], F16)
                if variant == "dma":
                    nc.vector.memset(ot[:], 0.0)
                    nc.scalar.dma_start(y[:, s * MO + m0 : s * MO + m0 + SB], ot[:])
                    continue

                c0 = 0  # sub-block offset within the super-block
                for sub in subs:
                    if variant != "pe" and last:
                        ot = op.tile([128, sub], F16)
                    pt = pp.tile([128, sub], F32)
                    for j in range(3):  # weight-outer: one LDWEIGHTS per j
                        xsrc = xo if j == 1 else xe
                        off = c0 + (1 if j == 2 else 0)
                        for q in range(sub // NSUB):
                            nc.tensor.matmul(
                                pt[:, q * NSUB : (q + 1) * NSUB],
                                wt[:, j * 128 : (j + 1) * 128],
                                xsrc[:, off + q * NSUB : off + q * NSUB + NSUB],
                                start=(j == 0),
                                stop=(j == 2),
                            )
                    if variant == "full":
                        dst = ot[:, c0 : c0 + sub] if not last else ot[:]
                        copy_engines[sub_idx % 2](dst, pt[:])
                        if last:
                            y0 = s * MO + m0 + c0
                            nc.scalar.dma_start(y[:, y0 : y0 + sub], ot[:])
                    sub_idx += 1
                    c0 += sub

                if variant != "pe" and not last:
                    y0 = s * MO + m0
                    nc.scalar.dma_start(y[:, y0 : y0 + SB], ot[:])
    nc.compile()
    return nc


def build_weights(kernels):
    """W [128, 3*128]: W[(c,p), j*128 + (o,r')] = ker_g[o, c, t] where
    t = p - r' (j=0, x_e), p + 4 - r' (j=1, x_o), p + 8 - r' (j=2, x_e+1)."""
    Wd = np.zeros((3, 128, 128), np.float32)
    for g, ker in enumerate(kernels):  # ker [4, 8, 5]
        for oo in range(4):
            o = 4 * g + oo
            for cc in range(8):
                c = 8 * g + cc
                for rp in range(8):
                    for p in range(4):
                        for j in range(3):
                            t = 4 * j + p - rp
                            if 0 <= t < W:
                                Wd[j, c * 4 + p, o * 8 + rp] = ker[oo, cc, t]
    return np.ascontiguousarray(
        np.concatenate([Wd[0], Wd[1], Wd[2]], axis=1)
    ).astype(np.float16)


def interleave_x(x1, dtype=np.float16):
    """[32, L] -> [128, MC]: per-sample [x_e | x_o] split-parity layout."""
    xp = np.zeros((D, L + 8), np.float32)
    xp[:, 2 : L + 2] = x1
    xr = xp.reshape(D, ME, 8)  # ME * 8 = L + 8
    xe = xr[:, :, 0:4].transpose(0, 2, 1).reshape(D * 4, ME)
    xo = xr[:, :MO, 4:8].transpose(0, 2, 1).reshape(D * 4, MO)
    return np.ascontiguousarray(
        np.concatenate([xe, xo], axis=1).astype(dtype)
    )


def deinterleave_y(yi):
    """[128, S*MO] fp16 -> [S*16, L] fp32: yi[o*8+r', s*MO+m] = y[s,o,8m+r']."""
    t = yi.reshape(16, 8, S, MO).transpose(2, 0, 3, 1)  # s, o, m, r'
    return np.ascontiguousarray(t.astype(np.float32).reshape(S * 16, L))


_program_cache = {}

# Set PROFILE=True (e.g. from a test harness) to capture an NTFF profile;
# the BassKernelResults lands in LAST_RESULT.
PROFILE = False
PROFILE_TMPDIR = None
LAST_RESULT = None


def kernel(batch_x, kernels0, kernels1, kernels2, kernels3):
    global LAST_RESULT
    batch_x = np.asarray(batch_x)
    kernels = [np.asarray(k) for k in (kernels0, kernels1, kernels2, kernels3)]
    Wd = build_weights(kernels)

    if "nc" not in _program_cache:
        _program_cache["nc"] = build_program()
    nc = _program_cache["nc"]

    in_maps = []
    for k in range(N_CORES):
        xs = [
            interleave_x(batch_x[S * k + s].reshape(D, L)) for s in range(S)
        ]
        in_maps.append({"x": np.concatenate(xs, axis=1), "w": Wd})

    res = run_bass_kernel_spmd(
        nc, in_maps, list(range(N_CORES)), trace=PROFILE, tmpdir=PROFILE_TMPDIR
    )
    LAST_RESULT = res
    ys = [deinterleave_y(res.results[k]["y"]) for k in range(N_CORES)]
    return np.concatenate(ys, axis=0).reshape(B, 16 * L)
